# revision 23
# baseline (speedup 1.0000x reference)
"""Trainium2 Bass kernel for nn_Decoder (Tacotron2-style decoder with
location-sensitive attention), data-parallel over batch across 8 NeuronCores.

Self-contained: hardcodes all shapes; builds + runs the Bass program via
concourse (available on sys.path in the runtime container).

Restructuring vs the reference (validated to 8e-7 rel in fp32):
 - conv(loc) + loc_proj folded into one [62, D] matmul over an im2col of
   shifted a/a_cum windows; q and the scalar bias row ride along as extra
   contraction rows (one-hot / ones rhs rows), so z = loc+q+bias lands in
   PSUM from a single matmul per tile.
 - mproj (enc @ mem_w^T + mem_b) precomputed on device; added by DVE.
 - prenet and the mel_in-dependent half of the att-LSTM input gates
   (pm @ Wih[:, :256]^T + biases) precomputed on device into DRAM.
 - gates reordered i|f|o|g so the three sigmoids are contiguous.
 - v_b dropped (softmax shift-invariant).
 - mel/stop fused into one [81, 512] output head.
"""
import numpy as np

# ---- problem shapes (hardcoded per spec) ----
B, T_ENC, T_DEC, D, MEL, PRE = 64, 300, 500, 512, 80, 256
KT, C_LOC = 31, 32
PAD = (KT - 1) // 2          # 15
NCORES = 8
BS = B // NCORES             # 8 examples per core
BT = BS * T_ENC              # 2400 attention columns per core
TCH = 3                      # ceil(300/128) t-chunks (padded to 384)
DCH = 4                      # 512/128 d-chunks
G = 4 * D                    # 2048 gates
PADW = T_ENC + 2 * PAD + 2   # 332 padded width for conv windows
ZW = 480                     # z/e column granule (5 per 2400)
PMW = 500                    # prepass matmul N granule

_CACHE = {}
LAST_EXEC_NS = None


def _f(x):
    return np.ascontiguousarray(np.asarray(x, np.float32))


def _bf(x):
    import ml_dtypes
    return np.ascontiguousarray(np.asarray(x).astype(ml_dtypes.bfloat16))


def _fold_params(inp):
    """Host-side parameter folding (params only, no input-tensor compute)."""
    p = {}

    def reorder(w):  # i,f,g,o -> i,f,o,g so sigmoid gates are contiguous
        i, f, g, o = np.split(_f(w), 4, axis=0)
        return np.concatenate([i, f, o, g], axis=0)

    att_wih = reorder(inp['att_wih']); att_whh = reorder(inp['att_whh'])
    att_b = reorder((_f(inp['att_bih']) + _f(inp['att_bhh']))[:, None])[:, 0]
    dec_wih = reorder(inp['dec_wih']); dec_whh = reorder(inp['dec_whh'])
    dec_b = reorder((_f(inp['dec_bih']) + _f(inp['dec_bhh']))[:, None])[:, 0]

    p['w1T'] = _bf(_f(inp['prenet_w1']).T)                       # [80, 256]
    p['b1'] = _f(inp['prenet_b1']).reshape(2, 128).T.copy()      # [128, 2]
    p['w2pT'] = _bf(_f(inp['prenet_w2']).T)                      # [256, 256]
    p['b2'] = _f(inp['prenet_b2']).reshape(2, 128).T.copy()
    p['wpmT'] = _bf(att_wih[:, :PRE].T)                          # [256, 2048]
    p['attb'] = att_b.reshape(16, 128).T.copy()                  # [128, 16]
    p['memwT'] = _bf(_f(inp['mem_w']).T)                         # [j, d]
    p['memb'] = _f(inp['mem_b']).reshape(4, 128).T.copy()        # [128, 4]
    p['wattT'] = _bf(np.concatenate([att_wih[:, PRE:], att_whh], axis=1).T)  # [1024, 2048]
    p['wdecT'] = _bf(np.concatenate([dec_wih, dec_whh], axis=1).T)           # [1536, 2048]
    p['decb'] = _bf(dec_b[None, :])                              # [1, 2048]
    p['qwT'] = _bf(_f(inp['q_w']).T)                             # [j, d]
    # location lhsT rows: 0:8 qT (per step), 8:39 a-taps, 39:70 acum-taps, 70 bias
    W2 = np.einsum('dc,cij->ijd', _f(inp['loc_proj_w']), _f(inp['loc_conv_w']))
    W2 = W2.reshape(2 * KT, D)
    bias_row = (_f(inp['loc_proj_b']) + _f(inp['loc_proj_w']) @ _f(inp['loc_conv_b'])
                + _f(inp['q_b']))
    locw = np.zeros((71, D), np.float32)
    locw[8:70] = W2
    locw[70] = bias_row
    p['locw'] = _bf(locw)
    p['vT'] = _bf(_f(inp['v_w']).T)                              # [512, 1]
    p['melwT'] = _bf(np.concatenate([_f(inp['mel_w']), _f(inp['stop_w'])], axis=0).T)
    p['melb'] = _bf(np.concatenate([_f(inp['mel_b']), _f(inp['stop_b'])])[None, :])
    onehot = np.zeros((9, BT), np.float32)
    for b in range(BS):
        onehot[b, b * T_ENC:(b + 1) * T_ENC] = 1.0
    onehot[8, :] = 1.0                                           # ones row (bias)
    p['im2c'] = _bf(onehot)
    p['id8'] = _f(np.eye(8))
    p['id128b'] = _bf(np.eye(128))
    p['ones8'] = _bf(np.ones((1, 8)))
    p['zpad'] = _bf(np.zeros((BS, PADW)))
    return p


def _build(T_steps, unroll, dbg=False):
    import concourse.bass as bass
    import concourse.mybir as mybir
    from concourse import bacc, tile

    dt = mybir.dt
    AF = mybir.ActivationFunctionType
    ALU = mybir.AluOpType
    AX = mybir.AxisListType
    f32, bf16 = dt.float32, dt.bfloat16
    ds = bass.ds

    nc = bacc.Bacc("TRN2", target_bir_lowering=False, debug=False,
                   num_devices=NCORES)

    def ein(name, shape, dtype):
        return nc.dram_tensor(name, shape, dtype, kind="ExternalInput")

    enc_t_d = ein("enc_t", [BS, TCH, 128, D], bf16)
    enc_d_d = ein("enc_d", [DCH, 128, BT], bf16)
    melT_d = ein("melT", [80, BS * T_steps], bf16)
    w1T_d = ein("w1T", [80, PRE], bf16)
    b1_d = ein("b1", [128, 2], f32)
    w2pT_d = ein("w2pT", [PRE, PRE], bf16)
    b2_d = ein("b2", [128, 2], f32)
    wpmT_d = ein("wpmT", [PRE, G], bf16)
    attb_d = ein("attb", [128, 16], f32)
    memwT_d = ein("memwT", [D, D], bf16)
    memb_d = ein("memb", [128, 4], f32)
    wattT_d = ein("wattT", [2 * D, G], bf16)
    wdecT_d = ein("wdecT", [3 * D, G], bf16)
    decb_d = ein("decb", [1, G], bf16)
    qwT_d = ein("qwT", [D, D], bf16)
    locw_d = ein("locw", [71, D], bf16)
    vT_d = ein("vT", [D, 1], bf16)
    melwT_d = ein("melwT", [D, MEL + 1], bf16)
    melb_d = ein("melb", [1, MEL + 1], bf16)
    im2c_d = ein("im2c", [9, BT], bf16)
    id8_d = ein("id8", [8, 8], f32)
    id128b_d = ein("id128b", [128, 128], bf16)
    ones8_d = ein("ones8", [1, 8], bf16)
    zpad_d = ein("zpad", [BS, PADW], bf16)

    mel_out_d = nc.dram_tensor("mel_stage", [T_steps * BS, MEL + 1], f32,
                               kind="ExternalOutput")
    dbg_d = {}
    if dbg:
        for nm, shape, dty in [("d_hatt", [8, D], f32), ("d_catt", [8, D], f32),
                               ("d_et", [8, T_ENC], f32), ("d_a", [8, T_ENC], f32),
                               ("d_gpm", [8, G], bf16), ("d_ctxT", [128, 4, 8], bf16),
                               ("d_mproj", [128, DCH, ZW], bf16),
                               ("d_im2", [71, ZW], bf16),
                               ("d_locT", [71, D], bf16),
                               ("d_hdec", [8, D], f32)]:
            dbg_d[nm] = nc.dram_tensor(nm, shape, dty, kind="ExternalOutput")

    with tile.TileContext(nc) as tc:
        from contextlib import ExitStack
        stack = ExitStack()
        persist = stack.enter_context(tc.tile_pool(name="persist", bufs=1))
        dramp = stack.enter_context(tc.tile_pool(name="dram", bufs=1, space="DRAM"))

        gpm_dram = dramp.tile([T_steps * BS, G], bf16, tag="gpm")
        apad_dram = dramp.tile([BS, PADW], bf16, tag="apad")
        cpad_dram = dramp.tile([BS, PADW], bf16, tag="cpad")

        # persistent small tiles + state
        mproj = persist.tile([128, DCH, BT], bf16, tag="mproj")
        locT = persist.tile([71, D], bf16, tag="locT")
        vT = persist.tile([128, 4], bf16, tag="vT")
        melwT = persist.tile([128, 4, MEL + 1], bf16, tag="melwT")
        melb = persist.tile([1, MEL + 1], bf16, tag="melb")
        decb = persist.tile([1, G], bf16, tag="decb")
        id8 = persist.tile([8, 8], f32, tag="id8")
        id8b = persist.tile([8, 8], bf16, tag="id8b")
        id128b = persist.tile([128, 128], bf16, tag="id128b")
        ones8 = persist.tile([1, 8], bf16, tag="ones8")
        im2A = persist.tile([71, BT], bf16, tag="im2A")
        im2B = persist.tile([71, BT], bf16, tag="im2B")
        h_att = persist.tile([8, D], f32, tag="h_att")
        c_att = persist.tile([8, D], f32, tag="c_att")
        h_dec = persist.tile([8, D], f32, tag="h_dec")
        c_dec = persist.tile([8, D], f32, tag="c_dec")
        a_sb = persist.tile([8, T_ENC], f32, tag="a_sb")
        acum = persist.tile([8, T_ENC], f32, tag="acum")
        a_bf = persist.tile([8, T_ENC], bf16, tag="a_bf")
        acum_bf = persist.tile([8, T_ENC], bf16, tag="acum_bf")
        hattT = persist.tile([128, 4, 8], bf16, tag="hattT")
        hdecT = persist.tile([128, 4, 8], bf16, tag="hdecT")
        ctxT = persist.tile([128, 4, 8], bf16, tag="ctxT")
        aT = persist.tile([128, TCH, 8], bf16, tag="aT")
        e_t = persist.tile([8, T_ENC], f32, tag="e_t")

        dma = nc.sync.dma_start

        # ---------------- constants + state init ----------------
        dma(id8[:, :], id8_d[:, :])
        dma(id128b[:, :], id128b_d[:, :])
        dma(ones8[:, :], ones8_d[:, :])
        dma(decb[:, :], decb_d[:, :])
        dma(melb[:, :], melb_d[:, :])
        dma(locT[:, :], locw_d[:, :])
        dma(vT[:, :], vT_d.rearrange("(c p) one -> p (c one)", p=128))
        dma(melwT[:, :, :], melwT_d.rearrange("(c p) m -> p c m", p=128))
        dma(im2A[0:8, :], im2c_d[0:8, :])
        dma(im2A[70:71, :], im2c_d[8:9, :])
        dma(im2B[0:8, :], im2c_d[0:8, :])
        dma(im2B[70:71, :], im2c_d[8:9, :])
        nc.vector.tensor_copy(id8b[:, :], id8[:, :])

        dma(apad_dram[:, :], zpad_d[:, :])
        dma(cpad_dram[:, :], zpad_d[:, :])
        for t_ in (h_att, c_att, h_dec, c_dec, a_sb, acum):
            nc.vector.memset(t_[:, :], 0.0)
        nc.vector.memset(hattT[:, :, :], 0.0)
        nc.vector.memset(ctxT[:, :, :], 0.0)
        nc.vector.memset(hdecT[:, :, :], 0.0)
        nc.vector.memset(aT[:, :, :], 0.0)

        # ---------------- prepass 1: prenet + gates_pm -> DRAM ----------------
        NB = BS * T_steps
        with (
            tc.tile_pool(name="pp", bufs=1) as pp,
            tc.tile_pool(name="pps", bufs=2) as pps,
            tc.tile_pool(name="ppp", bufs=3, space="PSUM") as ppp,
        ):
            melT = pp.tile([80, NB], bf16, tag="melT")
            dma(melT[:, :], melT_d[:, :])
            w1T = pp.tile([80, PRE], bf16, tag="w1T")
            dma(w1T[:, :], w1T_d[:, :])
            w2pT = pp.tile([128, 2, PRE], bf16, tag="w2pT")
            dma(w2pT[:, :, :], w2pT_d.rearrange("(c p) m -> p c m", p=128))
            wpmT = pp.tile([128, 2, G], bf16, tag="wpmT")
            dma(wpmT[:, :, :], wpmT_d.rearrange("(c p) m -> p c m", p=128))
            b1 = pp.tile([128, 2], f32, tag="b1"); dma(b1[:, :], b1_d[:, :])
            b2 = pp.tile([128, 2], f32, tag="b2"); dma(b2[:, :], b2_d[:, :])
            attb = pp.tile([128, 16], f32, tag="attb"); dma(attb[:, :], attb_d[:, :])
            h1 = pp.tile([128, 2, NB], bf16, tag="h1")
            pmt = pp.tile([128, 2, NB], bf16, tag="pmt")

            pmw = min(PMW, NB)
            nw = NB // pmw
            for fc in range(2):
                for ncol in range(nw):
                    ps = ppp.tile([128, pmw], f32, tag="pp")
                    nc.tensor.matmul(ps[:, :], w1T[:, fc * 128:(fc + 1) * 128],
                                     melT[:, ncol * pmw:(ncol + 1) * pmw],
                                     start=True, stop=True)
                    nc.scalar.activation(h1[:, fc, ncol * pmw:(ncol + 1) * pmw],
                                         ps[:, :], AF.Relu, bias=b1[:, fc:fc + 1])
            for fc in range(2):
                for ncol in range(nw):
                    ps = ppp.tile([128, pmw], f32, tag="pp")
                    for kc in range(2):
                        nc.tensor.matmul(ps[:, :], w2pT[:, kc, fc * 128:(fc + 1) * 128],
                                         h1[:, kc, ncol * pmw:(ncol + 1) * pmw],
                                         start=(kc == 0), stop=(kc == 1))
                    nc.scalar.activation(pmt[:, fc, ncol * pmw:(ncol + 1) * pmw],
                                         ps[:, :], AF.Relu, bias=b2[:, fc:fc + 1])
            gview = gpm_dram[:, :].rearrange("r (c p) -> p c r", p=128)
            for gc in range(16):
                for ncol in range(nw):
                    ps = ppp.tile([128, pmw], f32, tag="pp")
                    for kc in range(2):
                        nc.tensor.matmul(ps[:, :], wpmT[:, kc, gc * 128:(gc + 1) * 128],
                                         pmt[:, kc, ncol * pmw:(ncol + 1) * pmw],
                                         start=(kc == 0), stop=(kc == 1))
                    st = pps.tile([128, pmw], bf16, tag="gstage")
                    nc.scalar.activation(st[:, :], ps[:, :], AF.Identity,
                                         bias=attb[:, gc:gc + 1])
                    dma(gview[:, gc, ncol * pmw:(ncol + 1) * pmw], st[:, :])

        tc.strict_bb_all_engine_barrier()

        # ---------------- prepass 2: mproj ----------------
        with (
            tc.tile_pool(name="pm2", bufs=1) as pm2,
            tc.tile_pool(name="pm2p", bufs=3, space="PSUM") as pm2p,
        ):
            enc_dd = pm2.tile([128, DCH, BT], bf16, tag="enc_dd")
            dma(enc_dd[:, :, :], enc_d_d.rearrange("c p n -> p c n"))
            memwT = pm2.tile([128, 4, D], bf16, tag="memwT")
            dma(memwT[:, :, :], memwT_d.rearrange("(c p) m -> p c m", p=128))
            memb = pm2.tile([128, 4], f32, tag="memb"); dma(memb[:, :], memb_d[:, :])
            for dc in range(DCH):
                for ncol in range(BT // ZW):
                    ps = pm2p.tile([128, ZW], f32, tag="pm")
                    for jc in range(4):
                        nc.tensor.matmul(ps[:, :], memwT[:, jc, dc * 128:(dc + 1) * 128],
                                         enc_dd[:, jc, ncol * ZW:(ncol + 1) * ZW],
                                         start=(jc == 0), stop=(jc == 3))
                    nc.scalar.activation(mproj[:, dc, ncol * ZW:(ncol + 1) * ZW],
                                         ps[:, :], AF.Identity, bias=memb[:, dc:dc + 1])

        tc.strict_bb_all_engine_barrier()

        # ---------------- loop weights ----------------
        wattT = persist.tile([128, 8, G], bf16, tag="wattT")
        wdecT = persist.tile([128, 12, G], bf16, tag="wdecT")
        qwT = persist.tile([128, 4, D], bf16, tag="qwT")
        enc_t = persist.tile([128, BS * TCH, D], bf16, tag="enc_t")
        dma(wattT[:, :, :], wattT_d.rearrange("(c p) m -> p c m", p=128))
        dma(wdecT[:, :, :], wdecT_d.rearrange("(c p) m -> p c m", p=128))
        dma(qwT[:, :, :], qwT_d.rearrange("(c p) m -> p c m", p=128))
        dma(enc_t[:, :, :], enc_t_d.rearrange("b c p m -> p (b c) m"))

        # ---------------- loop pools ----------------
        lpA = stack.enter_context(tc.tile_pool(name="lpA", bufs=2))
        lpB = stack.enter_context(tc.tile_pool(name="lpB", bufs=6))
        lpC = stack.enter_context(tc.tile_pool(name="lpC", bufs=1))
        pgp = stack.enter_context(tc.tile_pool(name="pgp", bufs=2, space="PSUM"))
        pmisc = stack.enter_context(tc.tile_pool(name="pmisc", bufs=2, space="PSUM"))
        pzp = stack.enter_context(tc.tile_pool(name="pzp", bufs=2, space="PSUM"))
        pep = stack.enter_context(tc.tile_pool(name="pep", bufs=1, space="PSUM"))

        def lstm_elem(sg4, c_st, h_st):
            si, sf, so, sgg = sg4
            t1 = lpB.tile([8, D], f32, tag="sg")
            t2 = lpB.tile([8, D], f32, tag="sg")
            nc.vector.tensor_tensor(t1[:, :], sf[:, :], c_st[:, :], ALU.mult)
            nc.vector.tensor_tensor(t2[:, :], si[:, :], sgg[:, :], ALU.mult)
            nc.vector.tensor_tensor(c_st[:, :], t1[:, :], t2[:, :], ALU.add)
            tc_ = lpB.tile([8, D], f32, tag="sg")
            nc.scalar.activation(tc_[:, :], c_st[:, :], AF.Tanh)
            nc.vector.tensor_tensor(h_st[:, :], so[:, :], tc_[:, :], ALU.mult)

        def transpose8(src_ap, dst_ap, n_rows=128):
            pt = pmisc.tile([128, 8], f32, tag="pms")
            nc.tensor.transpose(pt[0:n_rows, :], src_ap, id8[:, :])
            nc.vector.tensor_copy(dst_ap, pt[0:n_rows, :])

        def gates(psum_tag, w_tile, lhs_list, extra, quarters_out):
            for qtr in range(4):
                pg = pgp.tile([8, 512], f32, tag="pg")
                nk = len(lhs_list)
                for kc in range(nk):
                    nc.tensor.matmul(pg[:, :], lhs_list[kc],
                                     w_tile[:, kc, qtr * 512:(qtr + 1) * 512],
                                     start=(kc == 0), stop=False)
                el, er = extra
                nc.tensor.matmul(pg[:, :], el, er[:, qtr * 512:(qtr + 1) * 512],
                                 start=False, stop=True)
                s = lpB.tile([8, 512], f32, tag="sg")
                nc.scalar.activation(s[:, :], pg[:, :],
                                     AF.Tanh if qtr == 3 else AF.Sigmoid)
                quarters_out.append(s)

        def step_body(tglob, im2):
            # --- prefetch gates_pm[t]; build im2col from a/acum of t-1 ---
            gpm = lpA.tile([8, G], bf16, tag="gpm")
            dma(gpm[:, :], gpm_dram[ds(tglob * BS, BS), :])
            ap0 = apad_dram[:, :]
            ap_src = bass.AP(ap0.tensor, ap0.offset, [[1, KT], [PADW, BS], [1, T_ENC]])
            cp0 = cpad_dram[:, :]
            cp_src = bass.AP(cp0.tensor, cp0.offset, [[1, KT], [PADW, BS], [1, T_ENC]])
            dma(im2[8:39, :].rearrange("p (b t) -> p b t", b=BS), ap_src)
            dma(im2[39:70, :].rearrange("p (b t) -> p b t", b=BS), cp_src)

            # --- att gates + LSTM ---
            sg4 = []
            gates("pg", wattT,
                  [ctxT[:, k, :] for k in range(4)] + [hattT[:, k, :] for k in range(4)],
                  (id8b[:, :], gpm), sg4)
            lstm_elem(sg4, c_att, h_att)
            for dc in range(DCH):
                transpose8(h_att[:, dc * 128:(dc + 1) * 128], hattT[:, dc, :])

            # --- q -> locT rows 0:8 ---
            pq = pgp.tile([8, 512], f32, tag="pg")
            for jc in range(4):
                nc.tensor.matmul(pq[:, :], hattT[:, jc, :], qwT[:, jc, :],
                                 start=(jc == 0), stop=(jc == 3))
            nc.scalar.activation(locT[0:8, :], pq[:, :], AF.Copy)

            # --- z pipe + e, granules of ZW cols ---
            pe1 = pep.tile([128, ZW], f32, tag="pe1")
            pe2 = pep.tile([128, ZW], f32, tag="pe2")
            for ncol in range(BT // ZW):
                pe, erow = (pe1, ncol) if ncol < 3 else (pe2, ncol - 3)
                cs = slice(ncol * ZW, (ncol + 1) * ZW)
                for dc in range(DCH):
                    pz = pzp.tile([128, ZW], f32, tag="pz")
                    nc.tensor.matmul(pz[:, :], locT[:, dc * 128:(dc + 1) * 128],
                                     im2[:, cs], start=True, stop=True)
                    zs = lpA.tile([128, ZW], bf16, tag="zs")
                    nc.vector.tensor_tensor(zs[:, :], pz[:, :], mproj[:, dc, cs],
                                            ALU.add)
                    zg = lpA.tile([128, ZW], bf16, tag="zg")
                    nc.scalar.activation(zg[:, :], zs[:, :], AF.Tanh)
                    nc.tensor.matmul(pe[32 * erow:32 * erow + 1, :], vT[:, dc:dc + 1],
                                     zg[:, :], start=(dc == 0), stop=(dc == 3),
                                     tile_position=(0, 32 * erow))
            es1 = lpA.tile([128, ZW], f32, tag="es")
            nc.vector.tensor_copy(es1[:, :], pe1[:, :])
            es2 = lpA.tile([128, ZW], f32, tag="es")
            nc.vector.tensor_copy(es2[:, :], pe2[:, :])
            # scatter e pieces -> e_t [8, 300]
            for ncol in range(BT // ZW):
                es, erow = (es1, ncol) if ncol < 3 else (es2, ncol - 3)
                lo = ncol * ZW
                while lo < (ncol + 1) * ZW:
                    b = lo // T_ENC
                    hi = min((b + 1) * T_ENC, (ncol + 1) * ZW)
                    dma(e_t[b:b + 1, lo - b * T_ENC:hi - b * T_ENC],
                        es[32 * erow:32 * erow + 1, lo - ncol * ZW:hi - ncol * ZW])
                    lo = hi

            # --- softmax over t ---
            nmx = lpC.tile([8, 1], f32, tag="nmx")
            nc.vector.tensor_reduce(nmx[:, :], e_t[:, :], AX.X, ALU.max, negate=True)
            ee = lpC.tile([8, T_ENC], f32, tag="ee")
            nc.scalar.activation(ee[:, :], e_t[:, :], AF.Exp, bias=nmx[:, 0:1])
            ssum = lpC.tile([8, 1], f32, tag="ssum")
            nc.vector.tensor_reduce(ssum[:, :], ee[:, :], AX.X, ALU.add)
            rr = lpC.tile([8, 1], f32, tag="rr")
            nc.vector.reciprocal(rr[:, :], ssum[:, :])
            nc.vector.tensor_scalar(a_sb[:, :], ee[:, :], rr[:, 0:1], None, ALU.mult)

            # --- acum update, pads out, aT ---
            nc.vector.tensor_tensor(acum[:, :], acum[:, :], a_sb[:, :], ALU.add)
            nc.scalar.activation(a_bf[:, :], a_sb[:, :], AF.Copy)
            nc.scalar.activation(acum_bf[:, :], acum[:, :], AF.Copy)
            dma(apad_dram[:, PAD:PAD + T_ENC], a_bf[:, :])
            dma(cpad_dram[:, PAD:PAD + T_ENC], acum_bf[:, :])
            for tcc in range(TCH):
                n_rows = min(128, T_ENC - tcc * 128)
                transpose8(a_sb[:, tcc * 128:tcc * 128 + n_rows],
                           aT[0:n_rows, tcc, :], n_rows=n_rows)

            # --- ctx = a . enc (col-packed psum rows per b) ---
            for grp in range(2):
                pc = pmisc.tile([128, 512], f32, tag="pms")
                for bl in range(4):
                    b = grp * 4 + bl
                    for tcc in range(TCH):
                        nc.tensor.matmul(pc[32 * bl:32 * bl + 1, :],
                                         aT[:, tcc, b:b + 1],
                                         enc_t[:, b * TCH + tcc, :],
                                         start=(tcc == 0), stop=(tcc == TCH - 1),
                                         tile_position=(0, 32 * bl))
                csp = lpA.tile([128, 512], bf16, tag="csp")
                nc.vector.tensor_copy(csp[:, :], pc[:, :])
                for dc in range(DCH):
                    pt2 = pmisc.tile([128, 128], bf16, tag="pms")
                    nc.tensor.transpose(pt2[:, :], csp[:, dc * 128:(dc + 1) * 128],
                                        id128b[:, :])
                    nc.vector.tensor_copy(ctxT[:, dc, grp * 4:grp * 4 + 4],
                                          pt2[:, 0:128:32])

            # --- dec gates + LSTM ---
            sd4 = []
            gates("pg", wdecT,
                  [hattT[:, k, :] for k in range(4)] + [ctxT[:, k, :] for k in range(4)]
                  + [hdecT[:, k, :] for k in range(4)],
                  (ones8[:, :], decb), sd4)
            lstm_elem(sd4, c_dec, h_dec)
            for dc in range(DCH):
                transpose8(h_dec[:, dc * 128:(dc + 1) * 128], hdecT[:, dc, :])

            # --- mel/stop head ---
            pm8 = pmisc.tile([8, MEL + 1], f32, tag="pms")
            for dc in range(DCH):
                nc.tensor.matmul(pm8[:, :], hdecT[:, dc, :], melwT[:, dc, :],
                                 start=(dc == 0), stop=False)
            nc.tensor.matmul(pm8[:, :], ones8[:, :], melb[:, :], start=False, stop=True)
            mel_sb = lpA.tile([8, MEL + 1], f32, tag="mel_sb")
            nc.vector.tensor_copy(mel_sb[:, :], pm8[:, :])
            dma(mel_out_d[ds(tglob * BS, BS), :], mel_sb[:, :])

            if dbg_d and tglob is not None and not hasattr(tglob, 'name') \
                    and tglob == T_steps - 1:
                dma(dbg_d["d_hatt"][:, :], h_att[:, :])
                dma(dbg_d["d_catt"][:, :], c_att[:, :])
                dma(dbg_d["d_et"][:, :], e_t[:, :])
                dma(dbg_d["d_a"][:, :], a_sb[:, :])
                dma(dbg_d["d_gpm"][:, :], gpm[:, :])
                dma(dbg_d["d_ctxT"][:, :, :], ctxT[:, :, :])
                dma(dbg_d["d_mproj"][:, :, :], mproj[:, :, 0:ZW])
                dma(dbg_d["d_im2"][:, :], im2[:, 0:ZW])
                dma(dbg_d["d_locT"][:, :], locT[:, :])
                dma(dbg_d["d_hdec"][:, :], h_dec[:, :])

        tc.strict_bb_all_engine_barrier()

        assert T_steps % unroll == 0
        if T_steps > unroll:
            with tc.For_i(0, T_steps, unroll) as it:
                for s in range(unroll):
                    step_body(it + s, im2A if s % 2 == 0 else im2B)
        else:
            for s in range(T_steps):
                step_body(s, im2A if s % 2 == 0 else im2B)

        stack.close()

    nc.compile()
    return nc


def _prep_inputs(inputs, T_steps):
    p = _fold_params(inputs)
    enc = _f(inputs['enc'])
    mel_in = _f(inputs['mel_in'])
    in_maps = []
    for k in range(NCORES):
        encs = enc[k * BS:(k + 1) * BS]                       # [8, 300, 512]
        mels = mel_in[k * BS:(k + 1) * BS, :T_steps]          # [8, T, 80]
        enc_pad = np.zeros((BS, TCH * 128, D), np.float32)
        enc_pad[:, :T_ENC] = encs
        m = dict(p)
        m['enc_t'] = _bf(enc_pad.reshape(BS, TCH, 128, D))
        m['enc_d'] = _bf(encs.reshape(BT, D).T.reshape(DCH, 128, BT))
        m['melT'] = _bf(mels.transpose(2, 1, 0).reshape(80, T_steps * BS))
        in_maps.append(m)
    return in_maps


def _run_timed(nc, in_maps, reps=3):
    """Run the bass program on 8 cores via PJRT with device-resident inputs,
    returning (per-core results, best wall-clock ns of a single execution)."""
    import time
    import jax
    import numpy as np
    from jax.experimental.shard_map import shard_map
    from jax.sharding import Mesh, PartitionSpec, NamedSharding
    from concourse import bass2jax, mybir
    bass2jax.install_neuronx_cc_hook()

    n_cores = len(in_maps)
    part_name = nc.partition_id_tensor.name if nc.partition_id_tensor else None
    in_names, out_names, out_avals, zero_outs = [], [], [], []
    for alloc in nc.m.functions[0].allocations:
        if not isinstance(alloc, mybir.MemoryLocationSet):
            continue
        name = alloc.memorylocations[0].name
        if alloc.kind == "ExternalInput":
            if name != part_name:
                in_names.append(name)
        elif alloc.kind == "ExternalOutput":
            shape = tuple(alloc.tensor_shape)
            dtype = mybir.dt.np(alloc.dtype)
            out_names.append(name)
            out_avals.append(jax.core.ShapedArray(shape, dtype))
            zero_outs.append(np.zeros(shape, dtype))
    n_params = len(in_names)
    n_outs = len(out_avals)
    all_names = in_names + out_names
    if part_name is not None:
        all_names = all_names + [part_name]

    def _body(*args):
        operands = list(args)
        if part_name is not None:
            operands.append(bass2jax.partition_id_tensor())
        outs = bass2jax._bass_exec_p.bind(
            *operands,
            out_avals=tuple(out_avals),
            in_names=tuple(all_names),
            out_names=tuple(out_names),
            lowering_input_output_aliases=(),
            sim_require_finite=True,
            sim_require_nnan=True,
            nc=nc,
        )
        return tuple(outs)

    devices = jax.devices()[:n_cores]
    mesh = Mesh(np.asarray(devices), ("core",))
    in_specs = (PartitionSpec("core"),) * (n_params + n_outs)
    out_specs = (PartitionSpec("core"),) * n_outs
    sharded = jax.jit(shard_map(_body, mesh=mesh, in_specs=in_specs,
                                out_specs=out_specs, check_rep=False),
                      keep_unused=True)
    sh = NamedSharding(mesh, PartitionSpec("core"))
    dev_in = [jax.device_put(
        np.concatenate([np.asarray(in_maps[c][nm]) for c in range(n_cores)], axis=0), sh)
        for nm in in_names]
    dev_zero = [jax.device_put(
        np.zeros((n_cores * z.shape[0], *z.shape[1:]), z.dtype), sh)
        for z in zero_outs]
    out = sharded(*dev_in, *dev_zero)
    jax.block_until_ready(out)
    best = None
    for _ in range(reps):
        t0 = time.perf_counter()
        o = sharded(*dev_in, *dev_zero)
        jax.block_until_ready(o)
        dt_ns = (time.perf_counter() - t0) * 1e9
        best = dt_ns if best is None else min(best, dt_ns)
    results = [
        {nm: np.asarray(out[i]).reshape(n_cores, *out_avals[i].shape)[c]
         for i, nm in enumerate(out_names)}
        for c in range(n_cores)
    ]
    return results, best


def kernel(T_steps=T_DEC, unroll=2, trace=False, bench=0, **inputs):
    import sys
    for path in ('/opt/trn_rl_repo', '/root/.axon_site/_ro/trn_rl_repo'):
        if path not in sys.path:
            sys.path.append(path)
    from concourse.bass_utils import run_bass_kernel_spmd

    key = (T_steps, unroll)
    if key not in _CACHE:
        _CACHE[key] = _build(T_steps, unroll)
    nc = _CACHE[key]

    in_maps = _prep_inputs(inputs, T_steps)
    global LAST_EXEC_NS
    if bench:
        results, best_ns = _run_timed(nc, in_maps, reps=bench)
        LAST_EXEC_NS = best_ns
    else:
        res = run_bass_kernel_spmd(nc, in_maps, core_ids=list(range(NCORES)),
                                   trace=trace)
        LAST_EXEC_NS = res.exec_time_ns
        results = res.results

    mel_out = np.zeros((B, T_steps, MEL), np.float32)
    stop_out = np.zeros((B, T_steps), np.float32)
    for k in range(NCORES):
        stage = results[k]['mel_stage'].reshape(T_steps, BS, MEL + 1)
        mel_out[k * BS:(k + 1) * BS] = stage[:, :, :MEL].transpose(1, 0, 2)
        stop_out[k * BS:(k + 1) * BS] = stage[:, :, MEL].T
    return mel_out, stop_out


if __name__ == '__main__':
    import sys
    sys.path.insert(0, '/root/problem')
    from reference import setup_inputs
    inputs = {k: np.asarray(v) for k, v in setup_inputs().items()}
    out = kernel(**inputs)
    print([o.shape for o in out])
